# revision 45
# baseline (speedup 1.0000x reference)
"""Single-head causal attention (B=4, T=4096, D=512, H=128) on 8 TRN2 NeuronCores.

Sharding: data-parallel over batch (4 batches x 2 cores). The two cores of a
batch split the 32 query tiles zig-zag style so causal work is balanced
(each core gets one long-context and one short-context tile per pair).
One SPMD program serves both core "types": per-slot k-tile counts are padded
to a shared compile-time schedule, and the causal boundary is applied with
per-core 0/1 mask tiles supplied as input data.

Per-core device program (all matmuls fp16 inputs -> fp32 PSUM):
  K^T = (wk^T @ x^T), V = x @ wv + bv, Q^T = (wq'^T @ xq^T), wq' = wq/sqrt(H)
  per slot group (4 slots, k-outer): S^T[k,q] = K^T_tile.T @ Q^T block
  P = exp(S^T) (no max subtraction: scores are O(5), fp16 holds exp fine),
  boundary tiles multiplied by 0/1 masks, O[q, 0:129] += P^T.T @ [V | 1]
  out = O[:, :128] * (1 / O[:, 128]).
The ones-column of V carries the softmax denominator through the same PSUM
accumulation, so no running max/sum bookkeeping is needed.
"""

import numpy as np
import ml_dtypes

B, T, D, H = 4, 4096, 512, 128
P = 128          # partitions / tile edge
DO = D // P      # contraction chunks (4)
NT = T // P      # k tiles per batch (32)
NS = 16          # query-tile slots per core
TQ = NS * P      # queries per core (2048)
KC = [32 - 2 * s for s in range(NS)]   # k-tiles processed per slot (desc)

_f16 = np.float16

_CACHE = {}


def _slot_qtile(core_type: int):
    """Global q-tile index handled by each slot, for core type 0/1."""
    out = []
    for s in range(NS):
        if s < 8:
            j = 31 - 2 * s - core_type          # long-context slots
        else:
            p = 15 - s
            j = 2 * p + core_type               # short-context slots
        out.append(j)
    return out


def _build_program():
    import concourse.tile as tile
    from concourse import bacc, mybir
    from concourse.bass import ts, ds

    f16 = mybir.dt.float16
    f32 = mybir.dt.float32
    Exp = mybir.ActivationFunctionType.Exp

    nc = bacc.Bacc("TRN2", target_bir_lowering=False, debug=False, num_devices=8)

    xT_d = nc.dram_tensor("xT", [P, DO, T], f16, kind="ExternalInput").ap()
    xqT_d = nc.dram_tensor("xqT", [P, DO, TQ], f16, kind="ExternalInput").ap()
    wq_d = nc.dram_tensor("wq", [P, DO, P], f16, kind="ExternalInput").ap()
    wk_d = nc.dram_tensor("wk", [P, DO, P], f16, kind="ExternalInput").ap()
    wv_d = nc.dram_tensor("wv", [P, DO, P], f16, kind="ExternalInput").ap()
    bq_d = nc.dram_tensor("bq", [P, 1], f32, kind="ExternalInput").ap()
    bk_d = nc.dram_tensor("bk", [P, 1], f32, kind="ExternalInput").ap()
    bvb_d = nc.dram_tensor("bvb", [P, P], f32, kind="ExternalInput").ap()
    tril_d = nc.dram_tensor("tril", [P, P], f16, kind="ExternalInput").ap()
    ab_d = nc.dram_tensor("ab", [P, NS * 2 * 2], f32, kind="ExternalInput").ap()
    out_d = nc.dram_tensor("out", [NS, P, P], f32, kind="ExternalOutput").ap()
    warm_d = nc.dram_tensor("warm", [P, 1], f32, kind="ExternalOutput").ap()

    NSTRIP = T // 512          # 8 key strips
    NQSTRIP = TQ // 512        # 4 query strips (one per slot group)

    with tile.TileContext(nc) as tc:
        with tc.tile_pool(name="const", bufs=1) as cpool, \
             tc.tile_pool(name="data", bufs=1) as dpool:
            wq_sb = cpool.tile([P, DO, P], f16)
            wk_sb = cpool.tile([P, DO, P], f16)
            wv_sb = cpool.tile([P, DO, P], f16)
            bq_sb = cpool.tile([P, 1], f32)
            bk_sb = cpool.tile([P, 1], f32)
            bvb_sb = cpool.tile([P, P], f32)
            msk_sb = cpool.tile([P, NS * 2 * P], f16)
            tril_sb = cpool.tile([P, P], f16)
            ab_sb = cpool.tile([P, NS * 2 * 2], f32)
            # constants on the GpSimd queue so the Sync/Scalar queues are free
            # to fire the big x-strip DMAs immediately.
            for sb, d in [(wq_sb, wq_d), (wk_sb, wk_d), (wv_sb, wv_d),
                          (bq_sb, bq_d), (bk_sb, bk_d), (bvb_sb, bvb_d),
                          (tril_sb, tril_d), (ab_sb, ab_d)]:
                nc.gpsimd.dma_start(sb[:], d)

            warm_sb = cpool.tile([P, 1], f32)
            nc.vector.memset(warm_sb[:], 0.0)

            # per-strip tiles so dependencies stay fine-grained: attention on
            # early k-tiles runs while later x strips are still in flight.
            xq_t = [dpool.tile([P, DO, 512], f16, name=f"xq_{i}")
                    for i in range(NQSTRIP)]
            xt_t = [dpool.tile([P, DO, 512], f16, name=f"xt_{i}")
                    for i in range(NSTRIP)]
            qt_t = [dpool.tile([P, 512], f16, name=f"qt_{i}")
                    for i in range(NQSTRIP)]
            kt_t = [dpool.tile([P, 512], f16, name=f"kt_{i}")
                    for i in range(NSTRIP)]
            v_t = [dpool.tile([P, 130], f16, name=f"v_{i}") for i in range(NT)]

            # x strips: query strips on the Scalar queue, key strips on Sync —
            # two parallel trigger streams, ordered by consumption. The first
            # query strip is split per contraction chunk so Q^T starts sooner.
            for o in range(DO):
                nc.scalar.dma_start(xq_t[0][:, o], xqT_d[:, o, ts(0, 512)])
            for st in range(1, NQSTRIP):
                nc.scalar.dma_start(xq_t[st][:], xqT_d[:, :, ts(st, 512)])
            for st in range(NSTRIP):
                nc.sync.dma_start(xt_t[st][:], xT_d[:, :, ts(st, 512)])
            nc.sync.dma_start(warm_d, warm_sb)
            for tt in range(NT):
                nc.gpsimd.memset(v_t[tt][:, 128:129], 1.0)

            with tc.tile_pool(name="ps_o", bufs=4, space="PSUM") as po_pool, \
                 tc.tile_pool(name="sb_w", bufs=6) as wpool, \
                 tc.tile_pool(name="sb_f", bufs=3) as fpool:

                def finalize(s, o_acc):
                    rec = fpool.tile([P, 1], f32, tag="rec", name=f"rec_{s}")
                    nc.vector.reciprocal(rec, o_acc[:, 128:129])
                    o_sb = fpool.tile([P, P], f32, tag="osb", name=f"osb_{s}")
                    nc.vector.tensor_scalar_mul(o_sb, o_acc[:, 0:128], rec)
                    nc.sync.dma_start(out_d[s], o_sb)

                def boundary_masks(p_ap_fn, u, slots, w):
                    for ci, s in enumerate(slots[:w]):
                        if u >= KC[s] - 2:
                            i = u - (KC[s] - 2)
                            nc.vector.tensor_mul(
                                p_ap_fn(ci), p_ap_fn(ci),
                                msk_sb[:, ds((2 * s + i) * P, P)])

                def proj_q(pp, st):
                    ps = pp.tile([P, 512], f32, tag="proj", name=f"psq_{st}")
                    for o in range(DO):
                        nc.tensor.matmul(ps, wq_sb[:, o], xq_t[st][:, o],
                                         start=(o == 0), stop=(o == DO - 1))
                    nc.vector.tensor_scalar_add(qt_t[st][:], ps, bq_sb)

                def proj_kv(pp, st):
                    ps = pp.tile([P, 512], f32, tag="proj", name=f"psk_{st}")
                    for o in range(DO):
                        nc.tensor.matmul(ps, wk_sb[:, o], xt_t[st][:, o],
                                         start=(o == 0), stop=(o == DO - 1))
                    nc.vector.tensor_scalar_add(kt_t[st][:], ps, bk_sb)
                    for j in range(4):
                        tt = 4 * st + j
                        ps_v = pp.tile([P, P], f32, tag="proj", name=f"psv_{tt}")
                        for o in range(DO):
                            nc.tensor.matmul(ps_v, xt_t[st][:, o, ts(j, P)],
                                             wv_sb[:, o],
                                             start=(o == 0), stop=(o == DO - 1))
                        nc.vector.tensor_add(v_t[tt][:, 0:128], ps_v, bvb_sb)

                def group_single(ps0, g):
                    """One k-tile per S/exp step (used while DMA still streams)."""
                    slots = list(range(4 * g, 4 * g + 4))
                    o_ps = {s: po_pool.tile([P, 129], f32, tag="oacc",
                                            name=f"o_acc_{s}") for s in slots}
                    for u in range(KC[slots[0]]):
                        w = sum(1 for s in slots if KC[s] > u)
                        s_sp = ps0.tile([P, 512], f32, tag="s0",
                                        name=f"s0_{g}_{u}")
                        nc.tensor.matmul(s_sp[:, 0:w * P],
                                         kt_t[u // 4][:, ts(u % 4, P)],
                                         qt_t[g][:, 0:w * P],
                                         start=True, stop=True)
                        p_sb = wpool.tile([P, 512], f16, tag="ptile0",
                                          name=f"p0_{g}_{u}")
                        nc.scalar.activation(p_sb[:, 0:w * P],
                                             s_sp[:, 0:w * P], Exp)
                        boundary_masks(lambda ci: p_sb[:, ts(ci, P)],
                                       u, slots, w)
                        for ci, s in enumerate(slots[:w]):
                            nc.tensor.matmul(o_ps[s], p_sb[:, ts(ci, P)],
                                             v_t[u][:, 0:129],
                                             start=(u == 0),
                                             stop=(u == KC[s] - 1))
                            if u == KC[s] - 1:
                                finalize(s, o_ps[s])

                # ---- phase 1: projections ----
                with tc.tile_pool(name="pproj", bufs=4, space="PSUM") as pp:
                    for st in range(NSTRIP):
                        if st < NQSTRIP:
                            proj_q(pp, st)
                        proj_kv(pp, st)

                # Boundary masks built on device: mask_blk = tril*b + a with
                # per-block (a, b) in {ones, tril, zeros} from tiny input data.
                # Emitted after projections so the DVE drains proj biases first
                # (masks are first needed at group 0's late boundary tiles).
                for blk in range(NS * 2):
                    nc.vector.tensor_scalar(
                        msk_sb[:, ts(blk, P)], tril_sb,
                        ab_sb[:, 2 * blk:2 * blk + 1],
                        ab_sb[:, 2 * blk + 1:2 * blk + 2],
                        mybir.AluOpType.mult, mybir.AluOpType.add)

                # ---- phase 2: attention, two k-tiles per exp ----
                with tc.tile_pool(name="ps_s", bufs=2, space="PSUM") as ps_pool:
                    for g in range(4):
                        slots = list(range(4 * g, 4 * g + 4))
                        o_ps = {s: po_pool.tile([P, 129], f32, tag="oacc",
                                                name=f"o_acc_{s}")
                                for s in slots}
                        c0 = KC[slots[0]]
                        for up in range(c0 // 2):      # k-tile pairs
                            u0 = 2 * up
                            w = sum(1 for s in slots if KC[s] > u0)
                            s_ps = ps_pool.tile([P, 2, 512], f32, tag="sacc",
                                                name=f"s_{g}_{up}")
                            for j in range(2):
                                u = u0 + j
                                nc.tensor.matmul(s_ps[:, j, 0:w * P],
                                                 kt_t[u // 4][:, ts(u % 4, P)],
                                                 qt_t[g][:, 0:w * P],
                                                 start=True, stop=True)
                            p_sb = wpool.tile([P, 2, 512], f16, tag="ptile",
                                              name=f"p_{g}_{up}")
                            if w == 4:      # contiguous AP: one clean ACT pass
                                nc.scalar.activation(
                                    p_sb.rearrange("p i q -> p (i q)"),
                                    s_ps.rearrange("p i q -> p (i q)"), Exp)
                            else:
                                nc.scalar.activation(p_sb[:, :, 0:w * P],
                                                     s_ps[:, :, 0:w * P], Exp)
                            for j in range(2):
                                boundary_masks(
                                    lambda ci, j=j: p_sb[:, j, ts(ci, P)],
                                    u0 + j, slots, w)
                            for j in range(2):
                                u = u0 + j
                                for ci, s in enumerate(slots[:w]):
                                    nc.tensor.matmul(o_ps[s],
                                                     p_sb[:, j, ts(ci, P)],
                                                     v_t[u][:, 0:129],
                                                     start=(u == 0),
                                                     stop=(u == KC[s] - 1))
                                    if u == KC[s] - 1:
                                        finalize(s, o_ps[s])

    nc.compile()
    return nc


def _prep_core(core, x, wq, bq, wk, bk, wv, bv):
    b, ct = core // 2, core % 2
    qtiles = _slot_qtile(ct)
    scale = np.float32(1.0 / np.sqrt(H))

    def dchunk(a):  # [D, N] -> [P, DO, N] with d = o*P + p
        return np.ascontiguousarray(
            a.reshape(DO, P, -1).transpose(1, 0, 2)).astype(_f16)

    xT = x[b].T.astype(np.float32)                      # [D, T]
    qrows = np.concatenate([np.arange(j * P, (j + 1) * P) for j in qtiles])
    xqT = np.ascontiguousarray(xT[:, qrows])            # [D, TQ]

    # per-block mask = tril*b + a: (a,b) = (1,0) ones / (0,1) tril / (0,0) zeros
    ab = np.zeros((P, NS * 2 * 2), dtype=np.float32)
    for s in range(NS):
        j = qtiles[s]
        for i in range(2):
            u = KC[s] - 2 + i
            blk = 2 * s + i
            if u < j:
                ab[:, 2 * blk + 1] = 1.0
            elif u == j:
                ab[:, 2 * blk] = 1.0

    return {
        "xT": dchunk(xT),
        "xqT": dchunk(xqT),
        "wq": dchunk(wq * scale),
        "wk": dchunk(wk),
        "wv": dchunk(wv),
        "bq": (bq * scale).astype(np.float32).reshape(P, 1),
        "bk": bk.astype(np.float32).reshape(P, 1),
        "bvb": np.tile(bv.astype(np.float32), (P, 1)),
        "tril": np.triu(np.ones((P, P), dtype=_f16)),
        "ab": ab,
    }


def _fallback(x, mask, wq, bq, wk, bk, wv, bv):
    """Exact numpy path for inputs the specialized kernel doesn't cover."""
    out = np.empty((x.shape[0], x.shape[1], wq.shape[1]), dtype=np.float32)
    scale = np.float32(1.0 / np.sqrt(wq.shape[1]))
    for b in range(x.shape[0]):
        q = x[b] @ wq + bq
        k = x[b] @ wk + bk
        v = x[b] @ wv + bv
        s = (q @ k.T) * scale
        s = np.where(mask == 0, np.float32(-1e30), s)
        s -= s.max(axis=-1, keepdims=True)
        p = np.exp(s)
        p /= p.sum(axis=-1, keepdims=True)
        out[b] = p @ v
    return out


def kernel(**inputs):
    x = np.asarray(inputs["x"], dtype=np.float32)
    mask = np.asarray(inputs["mask"])
    wq = np.asarray(inputs["wq"], dtype=np.float32)
    bq = np.asarray(inputs["bq"], dtype=np.float32)
    wk = np.asarray(inputs["wk"], dtype=np.float32)
    bk = np.asarray(inputs["bk"], dtype=np.float32)
    wv = np.asarray(inputs["wv"], dtype=np.float32)
    bv = np.asarray(inputs["bv"], dtype=np.float32)

    causal = (x.shape == (B, T, D) and wq.shape == (D, H)
              and np.array_equal(mask, np.tril(np.ones((T, T), mask.dtype))))
    if not causal:
        return _fallback(x, mask, wq, bq, wk, bk, wv, bv)

    if "nc" not in _CACHE:
        _CACHE["nc"] = _build_program()
    nc = _CACHE["nc"]

    from concourse import bass_utils
    in_maps = [_prep_core(c, x, wq, bq, wk, bk, wv, bv) for c in range(8)]
    res = bass_utils.run_bass_kernel_spmd(nc, in_maps, core_ids=list(range(8)),
                                          **_CACHE.get("run_kwargs", {}))
    _CACHE["last_result"] = res

    out = np.empty((B, T, H), dtype=np.float32)
    for c in range(8):
        b, ct = c // 2, c % 2
        qtiles = _slot_qtile(ct)
        oc = res.results[c]["out"]          # [NS, P, P]
        for s, j in enumerate(qtiles):
            out[b, j * P:(j + 1) * P, :] = oc[s]
    return out
